# revision 4
# baseline (speedup 1.0000x reference)
"""Trainium2 Bass kernel for nn_RNN2Classifier (Elman RNN H=2, T=4 + linear head).

Math (all weights are compile-time constants):
  h_t = tanh(W_ih x_t + b_ih + W_hh h_{t-1} + b_hh),  h_0 = 0
  out = cls_w . h_4 + cls_b

Factoring used (one scalar per op, matching scalar_tensor_tensor):
  s_th  = (x_t0 * wi[h,0]/wi[h,1]) + x_t1                    [STT]
  z_th  = (hp0 * wh[h,0]/wi[h,1]) + s_th ; z += hp1 * wh[h,1]/wi[h,1]
  h_th  = tanh(wi[h,1] * z + (bi[h]+bh[h]))                  [ACT, scale+bias free]
  q     = (h_30 * cw0/cw1) + h_31 ; out = Copy(cw1 * q + cb) [STT + ACT]

Sharding: pure data parallel, batch split 8 ways (500k rows/core).
Per-core layout: X viewed as [125 partitions, 4000 rows, 8 vals]; each
partition line is contiguous in DRAM.
"""

import sys

import numpy as np

sys.path.insert(0, "/opt/trn_rl_repo")

N_CORES = 8
B_TOTAL = 4_000_000
B_CORE = B_TOTAL // N_CORES  # 500_000
P = 125                      # partitions used (125 * 4000 = 500_000, no remainder)
G_TOTAL = B_CORE // P        # 4000 batch rows per partition
G = 400                      # chunk width (rows per partition per tile)
N_CHUNKS = G_TOTAL // G      # 10

WI = [[0.3519, -0.6514], [0.3238, 0.5568]]
BI = [0.2198, 0.4712]
WH = [[0.4279, 0.6832], [-0.4114, 0.5715]]
BH = [-0.409, -0.1299]
CW = [-0.2732, -0.1587]
CB = 0.5806

_cached_nc = None


def build_program():
    import concourse.bass as bass  # noqa: F401
    import concourse.tile as tile
    from concourse import bacc, mybir

    f32 = mybir.dt.float32
    mult = mybir.AluOpType.mult
    add = mybir.AluOpType.add
    Tanh = mybir.ActivationFunctionType.Tanh
    Copy = mybir.ActivationFunctionType.Copy

    r = [WI[0][0] / WI[0][1], WI[1][0] / WI[1][1]]
    a = [
        [WH[0][0] / WI[0][1], WH[0][1] / WI[0][1]],
        [WH[1][0] / WI[1][1], WH[1][1] / WI[1][1]],
    ]
    act_scale = [WI[0][1], WI[1][1]]
    act_bias = [BI[0] + BH[0], BI[1] + BH[1]]
    ccoef = CW[0] / CW[1]

    nc = bacc.Bacc(None, target_bir_lowering=False)

    # Tanh bias is lowered via the const-AP database; register our values.
    for val in act_bias:
        t = nc.alloc_sbuf_tensor(f"const-bias-{val}", [128, 1], f32)
        nc.gpsimd.memset(t.ap(), val)
        nc.const_aps.aps[(f32, val)] = t.ap()
    nc.all_engine_barrier()

    x_d = nc.dram_tensor("X", [B_CORE, 4, 2], f32, kind="ExternalInput")
    o_d = nc.dram_tensor("out", [B_CORE, 1], f32, kind="ExternalOutput")

    xv = x_d[:].rearrange("(p g) t d -> p (g t d)", p=P)  # [125, 32000]
    ov = o_d[:].rearrange("(p g) c -> p (g c)", p=P)      # [125, 4000]

    with tile.TileContext(nc) as tc:
        with (
            tc.tile_pool(name="io", bufs=2) as io_pool,
            tc.tile_pool(name="work", bufs=2) as work,
            tc.tile_pool(name="persist", bufs=1) as persist,
        ):
            out_acc = persist.tile([P, G_TOTAL], f32)
            for j in range(N_CHUNKS):
                xt = io_pool.tile([P, G * 8], f32)
                nc.sync.dma_start(out=xt, in_=xv[:, j * G * 8 : (j + 1) * G * 8])
                x4 = xt.rearrange("p (g t d) -> p g t d", t=4, d=2)  # [125,G,4,2]
                u = x4[:, :, :, 0]  # [125, G, 4], stride 8 / 2
                v = x4[:, :, :, 1]

                # input projections for all 4 timesteps in one op per h
                s = []
                for h in range(2):
                    sh = work.tile([P, 4 * G], f32)  # t-major segments of G
                    s3 = sh.rearrange("p (t g) -> p g t", t=4)  # iterate g out, t in
                    nc.vector.scalar_tensor_tensor(
                        out=s3, in0=u, scalar=r[h], in1=v, op0=mult, op1=add
                    )
                    s.append(sh)

                # t = 0
                hcur = []
                for h in range(2):
                    ht = work.tile([P, G], f32)
                    nc.scalar.activation(
                        out=ht,
                        in_=s[h][:, 0:G],
                        func=Tanh,
                        bias=act_bias[h],
                        scale=act_scale[h],
                    )
                    hcur.append(ht)

                # t = 1..3 ; h=0 chain: STT on DVE.
                # h=1 chain: ACT scale-copies + GPSIMD adds (Pool engine has
                # no scalar_tensor_tensor opcode on trn2).
                for t in range(1, 4):
                    hprev = hcur
                    hcur = []

                    tmp = work.tile([P, G], f32)
                    nc.vector.scalar_tensor_tensor(
                        out=tmp,
                        in0=hprev[0],
                        scalar=a[0][0],
                        in1=s[0][:, t * G : (t + 1) * G],
                        op0=mult,
                        op1=add,
                    )
                    z0 = work.tile([P, G], f32)
                    nc.vector.scalar_tensor_tensor(
                        out=z0,
                        in0=hprev[1],
                        scalar=a[0][1],
                        in1=tmp,
                        op0=mult,
                        op1=add,
                    )
                    h0 = work.tile([P, G], f32)
                    nc.scalar.activation(
                        out=h0, in_=z0, func=Tanh, bias=act_bias[0], scale=act_scale[0]
                    )
                    hcur.append(h0)

                    p0 = work.tile([P, G], f32)
                    nc.scalar.activation(
                        out=p0, in_=hprev[0], func=Copy, bias=0.0, scale=a[1][0]
                    )
                    p1 = work.tile([P, G], f32)
                    nc.scalar.activation(
                        out=p1, in_=hprev[1], func=Copy, bias=0.0, scale=a[1][1]
                    )
                    zz = work.tile([P, G], f32)
                    nc.gpsimd.tensor_add(zz, p0, s[1][:, t * G : (t + 1) * G])
                    z1 = work.tile([P, G], f32)
                    nc.gpsimd.tensor_add(z1, zz, p1)
                    h1 = work.tile([P, G], f32)
                    nc.scalar.activation(
                        out=h1, in_=z1, func=Tanh, bias=act_bias[1], scale=act_scale[1]
                    )
                    hcur.append(h1)

                # classifier
                pc = work.tile([P, G], f32)
                nc.scalar.activation(
                    out=pc, in_=hcur[0], func=Copy, bias=0.0, scale=ccoef
                )
                q = work.tile([P, G], f32)
                nc.gpsimd.tensor_add(q, pc, hcur[1])
                nc.scalar.activation(
                    out=out_acc[:, j * G : (j + 1) * G],
                    in_=q,
                    func=Copy,
                    bias=CB,
                    scale=CW[1],
                )

            nc.sync.dma_start(out=ov, in_=out_acc)

    nc.compile()
    return nc


def _get_nc():
    global _cached_nc
    if _cached_nc is None:
        _cached_nc = build_program()
    return _cached_nc


def run_sharded(X: np.ndarray, trace: bool = False):
    """Run the SPMD kernel on 8 cores. Returns (out_full, BassKernelResults)."""
    from concourse import bass_utils

    nc = _get_nc()
    X = np.ascontiguousarray(np.asarray(X, dtype=np.float32))
    assert X.shape == (B_TOTAL, 4, 2), X.shape
    in_maps = [
        {"X": X[i * B_CORE : (i + 1) * B_CORE]} for i in range(N_CORES)
    ]
    res = bass_utils.run_bass_kernel_spmd(
        nc, in_maps, core_ids=list(range(N_CORES)), trace=trace
    )
    out = np.concatenate([res.results[i]["out"] for i in range(N_CORES)], axis=0)
    return out, res


def kernel(**inputs: np.ndarray) -> np.ndarray:
    out, _ = run_sharded(inputs["X"])
    return out.astype(np.float32)


# revision 5
# speedup vs baseline: 1.3198x; 1.3198x over previous
"""Trainium2 Bass kernel for nn_RNN2Classifier (Elman RNN H=2, T=4 + linear head).

Math (all weights are compile-time constants):
  h_t = tanh(W_ih x_t + b_ih + W_hh h_{t-1} + b_hh),  h_0 = 0
  out = cls_w . h_4 + cls_b

Factoring used (one scalar per op, matching scalar_tensor_tensor):
  s_th  = (x_t0 * wi[h,0]/wi[h,1]) + x_t1                    [STT]
  z_th  = (hp0 * wh[h,0]/wi[h,1]) + s_th ; z += hp1 * wh[h,1]/wi[h,1]
  h_th  = tanh(wi[h,1] * z + (bi[h]+bh[h]))                  [ACT, scale+bias free]
  q     = (h_30 * cw0/cw1) + h_31 ; out = Copy(cw1 * q + cb) [STT + ACT]

Sharding: pure data parallel, batch split 8 ways (500k rows/core).
Per-core layout: X viewed as [125 partitions, 4000 rows, 8 vals]; each
partition line is contiguous in DRAM.
"""

import sys

import numpy as np

sys.path.insert(0, "/opt/trn_rl_repo")

N_CORES = 8
B_TOTAL = 4_000_000
B_CORE = B_TOTAL // N_CORES  # 500_000
P = 125                      # partitions used (125 * 4000 = 500_000, no remainder)
G_TOTAL = B_CORE // P        # 4000 batch rows per partition
G = 400                      # chunk width (rows per partition per tile)
N_CHUNKS = G_TOTAL // G      # 10

WI = [[0.3519, -0.6514], [0.3238, 0.5568]]
BI = [0.2198, 0.4712]
WH = [[0.4279, 0.6832], [-0.4114, 0.5715]]
BH = [-0.409, -0.1299]
CW = [-0.2732, -0.1587]
CB = 0.5806

_cached_nc = None


def build_program():
    import concourse.bass as bass  # noqa: F401
    import concourse.tile as tile
    from concourse import bacc, mybir

    f32 = mybir.dt.float32
    mult = mybir.AluOpType.mult
    add = mybir.AluOpType.add
    Tanh = mybir.ActivationFunctionType.Tanh
    Copy = mybir.ActivationFunctionType.Copy

    r = [WI[0][0] / WI[0][1], WI[1][0] / WI[1][1]]
    a = [
        [WH[0][0] / WI[0][1], WH[0][1] / WI[0][1]],
        [WH[1][0] / WI[1][1], WH[1][1] / WI[1][1]],
    ]
    act_scale = [WI[0][1], WI[1][1]]
    act_bias = [BI[0] + BH[0], BI[1] + BH[1]]
    ccoef = CW[0] / CW[1]

    nc = bacc.Bacc(None, target_bir_lowering=False)

    # Tanh bias is lowered via the const-AP database; register our values.
    for val in act_bias:
        t = nc.alloc_sbuf_tensor(f"const-bias-{val}", [128, 1], f32)
        nc.gpsimd.memset(t.ap(), val)
        nc.const_aps.aps[(f32, val)] = t.ap()
    nc.all_engine_barrier()

    x_d = nc.dram_tensor("X", [B_CORE, 4, 2], f32, kind="ExternalInput")
    o_d = nc.dram_tensor("out", [B_CORE, 1], f32, kind="ExternalOutput")

    xv = x_d[:].rearrange("(p g) t d -> p (g t d)", p=P)  # [125, 32000]
    ov = o_d[:].rearrange("(p g) c -> p (g c)", p=P)      # [125, 4000]

    with tile.TileContext(nc) as tc:
        with (
            tc.tile_pool(name="io", bufs=3) as io_pool,
            tc.tile_pool(name="work", bufs=2) as work,
            tc.tile_pool(name="persist", bufs=1) as persist,
        ):
            out_acc = persist.tile([P, G_TOTAL], f32, tag="out_acc")
            for j in range(N_CHUNKS):
                xt = io_pool.tile([P, G * 8], f32, tag="xt")
                nc.sync.dma_start(out=xt, in_=xv[:, j * G * 8 : (j + 1) * G * 8])
                x4 = xt.rearrange("p (g t d) -> p g t d", t=4, d=2)  # [125,G,4,2]
                u = x4[:, :, :, 0].rearrange("p g t -> p t g")  # [125,4,G] strided
                v = x4[:, :, :, 1].rearrange("p g t -> p t g")

                # input projections for all 4 timesteps in one op per h;
                # output iterates t-outer/g-inner so writes are contiguous
                s = []
                for h in range(2):
                    sh = work.tile([P, 4 * G], f32, tag=f"s{h}")
                    s3 = sh.rearrange("p (t g) -> p t g", t=4)
                    nc.vector.scalar_tensor_tensor(
                        out=s3, in0=u, scalar=r[h], in1=v, op0=mult, op1=add
                    )
                    s.append(sh)

                # t = 0
                hcur = []
                for h in range(2):
                    ht = work.tile([P, G], f32, tag=f"h{h}_t0")
                    nc.scalar.activation(
                        out=ht,
                        in_=s[h][:, 0:G],
                        func=Tanh,
                        bias=act_bias[h],
                        scale=act_scale[h],
                    )
                    hcur.append(ht)

                # t = 1..3 ; both h chains on DVE (2 STT each) + ACT tanh
                for t in range(1, 4):
                    hprev = hcur
                    hcur = []
                    for h in range(2):
                        tmp = work.tile([P, G], f32, tag=f"tmp{h}_t{t}")
                        nc.vector.scalar_tensor_tensor(
                            out=tmp,
                            in0=hprev[0],
                            scalar=a[h][0],
                            in1=s[h][:, t * G : (t + 1) * G],
                            op0=mult,
                            op1=add,
                        )
                        z = work.tile([P, G], f32, tag=f"z{h}_t{t}")
                        nc.vector.scalar_tensor_tensor(
                            out=z,
                            in0=hprev[1],
                            scalar=a[h][1],
                            in1=tmp,
                            op0=mult,
                            op1=add,
                        )
                        ht = work.tile([P, G], f32, tag=f"h{h}_t{t}")
                        nc.scalar.activation(
                            out=ht,
                            in_=z,
                            func=Tanh,
                            bias=act_bias[h],
                            scale=act_scale[h],
                        )
                        hcur.append(ht)

                # classifier
                q = work.tile([P, G], f32, tag="q")
                nc.vector.scalar_tensor_tensor(
                    out=q,
                    in0=hcur[0],
                    scalar=ccoef,
                    in1=hcur[1],
                    op0=mult,
                    op1=add,
                )
                nc.scalar.activation(
                    out=out_acc[:, j * G : (j + 1) * G],
                    in_=q,
                    func=Copy,
                    bias=CB,
                    scale=CW[1],
                )

            nc.sync.dma_start(out=ov, in_=out_acc)

    nc.compile()
    return nc


def _get_nc():
    global _cached_nc
    if _cached_nc is None:
        _cached_nc = build_program()
    return _cached_nc


def run_sharded(X: np.ndarray, trace: bool = False):
    """Run the SPMD kernel on 8 cores. Returns (out_full, BassKernelResults)."""
    from concourse import bass_utils

    nc = _get_nc()
    X = np.ascontiguousarray(np.asarray(X, dtype=np.float32))
    assert X.shape == (B_TOTAL, 4, 2), X.shape
    in_maps = [
        {"X": X[i * B_CORE : (i + 1) * B_CORE]} for i in range(N_CORES)
    ]
    res = bass_utils.run_bass_kernel_spmd(
        nc, in_maps, core_ids=list(range(N_CORES)), trace=trace
    )
    out = np.concatenate([res.results[i]["out"] for i in range(N_CORES)], axis=0)
    return out, res


def kernel(**inputs: np.ndarray) -> np.ndarray:
    out, _ = run_sharded(inputs["X"])
    return out.astype(np.float32)
